# revision 1
# baseline (speedup 1.0000x reference)
"""Trainium2 Bass kernel for nn_MemoryAugmented (scatter_memory).

Computes, for full inputs x:[64,12,883,64], M:[12,64,64]:
    score = softmax(einsum('blnd,tmd->btnm', x, M), axis=-1)
    out   = einsum('btnm,tmd->btnd', score, M)

Distribution: data-parallel over batch across 8 NeuronCores (8 batches
per core); the small memory bank M is replicated (sent pre-transformed
into two block-diagonal constant tensors so pairs of t share one
full-width 128-K matmul).

Per-core dataflow (rows r = (b, n), padded to 7*128 per batch):
  phase A  x[b,:,ntile,:] --DMA--> [P,12,64] --DVE tree-add over l-->
           xs [P,64] --PE transpose--> xsT --ACT copy x2--> xsT2 [128,512]
           (rows 0:64 and 64:128 both hold xs^T: K-replication for mm1)
  phase B  mm1: blockdiag(M[2tp]^T, M[2tp+1]^T)^T @ xsT2 -> logits
           [(2t x m)=128, 512] in PSUM; ACT exp (no max subtraction --
           |logits| < ~30, safe in fp32); mm2: exp_chunk^T @
           [blockdiag(M) | ones cols] -> [rows=128, (t0 d | t1 d | sums)]
           in PSUM; DVE reciprocal of sums + broadcast multiply
           normalizes and evacuates PSUM; one DMA per 128-row chunk
           writes out[b, :, nrange, :].
"""
import sys

for _p in ("/opt/trn_rl_repo",):
    if _p not in sys.path:
        sys.path.insert(0, _p)

from contextlib import ExitStack

import numpy as np

import concourse.bass as bass
import concourse.bacc as bacc
import concourse.tile as tile
from concourse import mybir
from concourse._compat import with_exitstack
from concourse.bass_utils import run_bass_kernel_spmd

B, L, N, D = 64, 12, 883, 64
T, MNUM = 12, 64
NCORES = 8
BS = B // NCORES          # 8 batches per core
NT = 7                    # n-tiles per batch: 6*128 + 115
G = BS * NT               # 56 row-chunks per core
NTILES = G // 4           # 14 tiles of 512 rows
F32 = mybir.dt.float32
F32R = mybir.dt.float32r
BF16 = mybir.dt.bfloat16


def build_consts(M):
    """Host-side layout prep (pure data movement) of the memory bank."""
    M = np.asarray(M, dtype=np.float32)
    mt2 = np.zeros((128, 6 * 128), np.float32)
    mbd = np.zeros((128, 6 * 130), np.float32)
    for tp in range(6):
        t0, t1 = 2 * tp, 2 * tp + 1
        mt2[0:64, tp * 128 + 0:tp * 128 + 64] = M[t0].T
        mt2[64:128, tp * 128 + 64:tp * 128 + 128] = M[t1].T
        mbd[0:64, tp * 130 + 0:tp * 130 + 64] = M[t0]
        mbd[64:128, tp * 130 + 64:tp * 130 + 128] = M[t1]
        mbd[0:64, tp * 130 + 128] = 1.0
        mbd[64:128, tp * 130 + 129] = 1.0
    eye = np.eye(128, dtype=np.float32)
    return mt2, mbd, eye


@with_exitstack
def kernel_body(ctx: ExitStack, tc: "tile.TileContext", out: bass.AP,
                x: bass.AP, mt2: bass.AP, mbd: bass.AP, eye: bass.AP):
    nc = tc.nc
    consts = ctx.enter_context(tc.tile_pool(name="consts", bufs=1))
    work = ctx.enter_context(tc.tile_pool(name="work", bufs=2))
    psum = ctx.enter_context(tc.tile_pool(name="psum", bufs=1, space="PSUM"))

    # const loads ride the scalar HWDGE ring (idle at kernel start) so the
    # first x-load isn't queued behind them on the sync ring's FIFO.
    mt2_sb = consts.tile([128, 6 * 128], F32)
    nc.scalar.dma_start(out=mt2_sb[:], in_=mt2[:])
    mbd_sb = consts.tile([128, 6 * 130], F32)
    nc.scalar.dma_start(out=mbd_sb[:], in_=mbd[:])
    eye_sb = consts.tile([128, 128], F32)
    nc.scalar.dma_start(out=eye_sb[:], in_=eye[:])
    zbias = consts.tile([128, 1], F32)
    nc.vector.memset(zbias[:], 0.0)

    for ti in range(NTILES):
        xsT = work.tile([128, 512], F32, tag="xsT", bufs=3)
        metas = []
        for c in range(4):
            g = ti * 4 + c
            b, nt = divmod(g, NT)
            n0 = nt * 128
            P = 128 if nt < NT - 1 else N - n0
            metas.append((b, n0, P))
        # two 768 KB DMAs per tile; l-sum tree runs two chunks per
        # instruction (quarter the op count of per-chunk trees, finer
        # overlap than one tile-wide load)
        for hh in range(2):
            xt = work.tile([128, 2 * L * D], F32, tag="xt", bufs=4)
            r0 = 512 * ti + 256 * hh
            nc.sync.dma_start(
                out=xt[:].rearrange("p (c f) -> p c f", c=2),
                in_=x[r0:r0 + 256, :, :]
                    .rearrange("(c p) l d -> p c (l d)", c=2),
            )
            t384 = work.tile([128, 2 * 384], F32, tag="t384", bufs=2)
            xtv = xt[:].rearrange("p (c h f) -> p c h f", c=2, h=2)
            nc.vector.tensor_add(t384[:].rearrange("p (c f) -> p c f", c=2),
                                 xtv[:, :, 0], xtv[:, :, 1])
            t192 = work.tile([128, 2 * 192], F32, tag="t192", bufs=2)
            t384v = t384[:].rearrange("p (c h f) -> p c h f", c=2, h=2)
            nc.vector.tensor_add(t192[:].rearrange("p (c f) -> p c f", c=2),
                                 t384v[:, :, 0], t384v[:, :, 1])
            t192v = t192[:].rearrange("p (c g f) -> p c g f", c=2, g=3)
            xs2 = work.tile([128, 2 * 64], F32, tag="xs2", bufs=2)
            xs2v = xs2[:].rearrange("p (c f) -> p c f", c=2)
            nc.vector.tensor_add(xs2v, t192v[:, :, 0], t192v[:, :, 1])
            xs4 = work.tile([128, 2 * 64], F32, tag="xs4", bufs=2)
            nc.vector.tensor_add(xs4[:].rearrange("p (c f) -> p c f", c=2),
                                 xs2v, t192v[:, :, 2])
            for cc in range(2):
                c = 2 * hh + cc
                ps_xsT = psum.tile([64, 128], F32, tag="ps_xsT", bufs=2)
                nc.tensor.transpose(ps_xsT[:], xs4[:, cc * 64:(cc + 1) * 64],
                                    eye_sb[:])
                cs = slice(c * 128, (c + 1) * 128)
                nc.scalar.copy(xsT[0:64, cs], ps_xsT[:])
                # K-replica for the blockdiag mm1; gpsimd is otherwise idle
                # and SBUF->SBUF is legal there (PSUM is not).
                nc.gpsimd.tensor_copy(xsT[64:128, cs], xsT[0:64, cs])

        exps = []
        for tp in range(6):
            ps_log = psum.tile([128, 512], F32, tag="logits", bufs=2)
            nc.tensor.matmul(ps_log[:], mt2_sb[:, tp * 128:(tp + 1) * 128],
                             xsT[:], start=True, stop=True)
            ex = work.tile([128, 512], F32, tag="exp", bufs=16)
            nc.scalar.activation(ex[:], ps_log[:],
                                 mybir.ActivationFunctionType.Exp, bias=zbias[:])
            exps.append(ex)

        for c in range(4):
            b, n0, P = metas[c]
            ps_val = psum.tile([128, 1024], F32, tag="val", bufs=2)
            for tp in range(6):
                off = 512 * (tp // 3) + 130 * (tp % 3)
                nc.tensor.matmul(ps_val[:, off:off + 130],
                                 exps[tp][:, c * 128:(c + 1) * 128],
                                 mbd_sb[:, tp * 130:(tp + 1) * 130],
                                 start=True, stop=True)
            # sums sit at free offsets {512h + 130a + 128 + t2}; one strided
            # reciprocal covers all 12.
            sums_ap = (ps_val[:].rearrange("p (h r) -> p h r", h=2)
                       [:, :, 0:390]
                       .rearrange("p h (a r) -> p h a r", a=3)
                       [:, :, :, 128:130])
            rec = work.tile([128, 12], F32, tag="rec", bufs=4)
            nc.vector.reciprocal(
                rec[:].rearrange("p (h a t) -> p h a t", h=2, a=3), sums_ap)
            vn = work.tile([128, T * D], F32, tag="vn", bufs=10)
            for h in range(2):
                in0 = (ps_val[:, 512 * h:512 * h + 390]
                       .rearrange("p (a r) -> p a r", a=3)
                       [:, :, 0:128]
                       .rearrange("p a (t d) -> p a t d", t=2))
                in1 = (rec[:, 6 * h:6 * h + 6]
                       .rearrange("p (a t) -> p a t", a=3)
                       .unsqueeze(3)
                       .broadcast_to([128, 3, 2, D]))
                outp = (vn[:, 384 * h:384 * h + 384]
                        .rearrange("p (a t d) -> p a t d", a=3, t=2))
                nc.vector.tensor_mul(outp, in0, in1)
            # stores go out on the ACT HWDGE ring so loads (sync ring) and
            # stores generate descriptors in parallel.
            nc.scalar.dma_start(
                out=out[b, n0:n0 + P, :, :].rearrange("n t d -> n (t d)"),
                in_=vn[:P],
            )


_NC_CACHE = {}


def build_nc():
    if "nc" in _NC_CACHE:
        return _NC_CACHE["nc"]
    nc = bacc.Bacc("TRN2", target_bir_lowering=False, debug=False,
                   num_devices=NCORES)
    # x is pre-transposed on the host to [BS, N, L, D], n-padded to 896 rows
    # per batch with zeros, and flattened to [7168, 12, 64]; the output is
    # produced as [BS, N, T, D]. Per-partition DMA runs become 3 KB
    # contiguous instead of 12x256 B (descriptor-rate-bound ~175 GB/s vs
    # HBM-bound ~358 GB/s), the whole 512-row tile arrives in one DMA, and
    # every chunk is a full 128 rows so the l-sum tree runs tile-wide.
    x_ap = nc.dram_tensor("x_sh", [BS * 896, L, D], F32, kind="ExternalInput").ap()
    mt2_ap = nc.dram_tensor("mt2", [128, 6 * 128], F32, kind="ExternalInput").ap()
    mbd_ap = nc.dram_tensor("mbd", [128, 6 * 130], F32, kind="ExternalInput").ap()
    eye_ap = nc.dram_tensor("eye", [128, 128], F32, kind="ExternalInput").ap()
    out_ap = nc.dram_tensor("out", [BS, N, T, D], F32, kind="ExternalOutput").ap()
    with tile.TileContext(nc) as tc:
        kernel_body(tc, out_ap, x_ap, mt2_ap, mbd_ap, eye_ap)
    nc.compile()
    _NC_CACHE["nc"] = nc
    return nc


def make_in_maps(x, M):
    x = np.asarray(x, dtype=np.float32)
    mt2, mbd, eye = build_consts(M)
    maps = []
    for i in range(NCORES):
        xp = np.zeros((BS, 896, L, D), np.float32)
        xp[:, :N] = x[i * BS:(i + 1) * BS].transpose(0, 2, 1, 3)
        maps.append({"x_sh": xp.reshape(BS * 896, L, D),
                     "mt2": mt2, "mbd": mbd, "eye": eye})
    return maps


def kernel(x, M):
    nc = build_nc()
    in_maps = make_in_maps(x, M)
    res = run_bass_kernel_spmd(nc, in_maps, list(range(NCORES))).results
    return np.ascontiguousarray(np.concatenate(
        [res[i]["out"].transpose(0, 2, 1, 3) for i in range(NCORES)], axis=0))


if __name__ == "__main__":
    rng = np.random.default_rng(0)
    x = rng.standard_normal((B, L, N, D), dtype=np.float32)
    M = (rng.standard_normal((T, MNUM, D), dtype=np.float32) * 0.125).astype(np.float32)
    out = kernel(x, M)
    print("out", out.shape, out.dtype, float(np.abs(out).max()))



# revision 5
# speedup vs baseline: 1.0463x; 1.0463x over previous
"""Trainium2 Bass kernel for nn_MemoryAugmented (scatter_memory).

Computes, for full inputs x:[64,12,883,64], M:[12,64,64]:
    score = softmax(einsum('blnd,tmd->btnm', x, M), axis=-1)
    out   = einsum('btnm,tmd->btnd', score, M)

Distribution: data-parallel over batch across 8 NeuronCores (8 batches
per core); the memory bank M is replicated, shipped pre-transformed into
two constant matrices (paired-t M^T for mm1, block-diagonal M + ones
columns for mm2's fused row sums).

Precision: x and M travel as fp16 (matmuls run at 1 cycle/row vs 4 for
fp32, HBM traffic halves); exp values are bf16 (need fp32-like range);
PSUM accumulation is always fp32; output is stored fp16 and upcast on
the host. Measured end-to-end max rel err ~4e-3 vs the 2e-2 gate.

Per-core dataflow, 7 iterations of 1024 rows r = (b, n):
  load   one DMA [128, 8*768] fp16; partition p holds rows 8p..8p+7
         (12 KB contiguous per partition line)
  l-sum  4-level pairwise tree: L1+L2 on gpsimd, L3+L4 on DVE (fp16)
  xsT    8 PE transposes -> PSUM, 2 ACT copies -> [64, 1024] fp16
  mm1    12 matmuls (6 t-pairs x 2 halves) K=64: logits [128tm, 512] PSUM
  exp    ACT Exp -> bf16 SBUF (no max subtraction; |logits| small)
  mm2    per 128-row chunk: 6 matmuls exp^T @ blockdiag(M)|ones -> value
         and row sums in one PSUM tile
  norm   DVE reciprocal of sums + broadcast multiply -> vn fp16
  store  one DMA [128, 8*768] fp16 per iteration
"""
import sys

for _p in ("/opt/trn_rl_repo",):
    if _p not in sys.path:
        sys.path.insert(0, _p)

from contextlib import ExitStack

import numpy as np

import concourse.bass as bass
import concourse.bacc as bacc
import concourse.tile as tile
from concourse import mybir
from concourse._compat import with_exitstack
from concourse.bass_utils import run_bass_kernel_spmd

B, L, N, D = 64, 12, 883, 64
T, MNUM = 12, 64
NCORES = 8
BS = B // NCORES          # 8 batches per core
NPAD = 896                # per-batch row pad (7*128)
ROWS = BS * NPAD          # 7168 rows per core
NIT = 7                   # iterations of 1024 rows
F32 = mybir.dt.float32
F16 = mybir.dt.float16
BF16 = mybir.dt.bfloat16


def build_consts(M):
    """Host-side layout prep (pure data movement) of the memory bank."""
    M = np.asarray(M, dtype=np.float32)
    mt2h = np.zeros((64, 6 * 128), np.float16)   # [d, (tp, q, m)] = M[2tp+q].T
    mbd = np.zeros((128, 6 * 130), np.float32)   # [(q, m), (tp, q, d | sums)]
    for tp in range(6):
        t0, t1 = 2 * tp, 2 * tp + 1
        mt2h[:, tp * 128 + 0:tp * 128 + 64] = M[t0].T.astype(np.float16)
        mt2h[:, tp * 128 + 64:tp * 128 + 128] = M[t1].T.astype(np.float16)
        mbd[0:64, tp * 130 + 0:tp * 130 + 64] = M[t0]
        mbd[64:128, tp * 130 + 64:tp * 130 + 128] = M[t1]
        mbd[0:64, tp * 130 + 128] = 1.0
        mbd[64:128, tp * 130 + 129] = 1.0
    eye = np.eye(128, dtype=np.float16)
    return mt2h, mbd, eye


@with_exitstack
def kernel_body(ctx: ExitStack, tc: "tile.TileContext", out: bass.AP,
                x: bass.AP, mt2h: bass.AP, mbd: bass.AP, eye: bass.AP):
    nc = tc.nc
    consts = ctx.enter_context(tc.tile_pool(name="consts", bufs=1))
    work = ctx.enter_context(tc.tile_pool(name="work", bufs=2))
    psum = ctx.enter_context(tc.tile_pool(name="psum", bufs=1, space="PSUM"))

    # const loads ride the scalar HWDGE ring (idle at kernel start) so the
    # first x-load isn't queued behind them on the sync ring's FIFO.
    mt2h_sb = consts.tile([64, 6 * 128], F16)
    nc.scalar.dma_start(out=mt2h_sb[:], in_=mt2h[:])
    mbd_sb = consts.tile([128, 6 * 130], BF16)
    nc.scalar.dma_start(out=mbd_sb[:], in_=mbd[:])
    eye_sb = consts.tile([128, 128], F16)
    nc.scalar.dma_start(out=eye_sb[:], in_=eye[:])
    zbias = consts.tile([128, 1], F32)
    nc.vector.memset(zbias[:], 0.0)

    for it in range(NIT):
        r0 = 1024 * it
        # one 1.5 MB load; partition p <- rows 8p..8p+7 (12 KB contiguous)
        xt = work.tile([128, 8 * L * D], F16, tag="xt", bufs=2)
        nc.sync.dma_start(
            out=xt[:].rearrange("p (c f) -> p c f", c=8),
            in_=x[r0:r0 + 1024, :, :]
                .rearrange("(p c) l d -> p c (l d)", c=8),
        )
        # l-sum tree: 12 -> 6 -> 3 -> (2 -> 1). The two big levels run on
        # gpsimd (otherwise idle), the small ones on DVE in fp16 2x mode.
        t384 = work.tile([128, 8 * 384], F16, tag="t384", bufs=2)
        xtv = xt[:].rearrange("p (c h f) -> p c h f", c=8, h=2)
        nc.gpsimd.tensor_add(t384[:].rearrange("p (c f) -> p c f", c=8),
                             xtv[:, :, 0], xtv[:, :, 1])
        t192 = work.tile([128, 8 * 192], F16, tag="t192", bufs=2)
        t384v = t384[:].rearrange("p (c h f) -> p c h f", c=8, h=2)
        nc.gpsimd.tensor_add(t192[:].rearrange("p (c f) -> p c f", c=8),
                             t384v[:, :, 0], t384v[:, :, 1])
        t192v = t192[:].rearrange("p (c g f) -> p c g f", c=8, g=3)
        xs2 = work.tile([128, 8 * 64], F16, tag="xs2", bufs=2)
        xs2v = xs2[:].rearrange("p (c f) -> p c f", c=8)
        nc.vector.tensor_add(xs2v, t192v[:, :, 0], t192v[:, :, 1])
        xs4 = work.tile([128, 8 * 64], F16, tag="xs4", bufs=2)
        nc.vector.tensor_add(xs4[:].rearrange("p (c f) -> p c f", c=8),
                             xs2v, t192v[:, :, 2])

        # transpose the 8 chunks to xsT [64 d, 1024 n] via PE + ACT copy
        xsT = work.tile([64, 1024], F16, tag="xsT", bufs=2)
        for half in range(2):
            ps_x = psum.tile([64, 512], F16, tag="ps_x", bufs=2)
            for cc in range(4):
                c = 4 * half + cc
                nc.tensor.transpose(ps_x[:, cc * 128:(cc + 1) * 128],
                                    xs4[:, c * 64:(c + 1) * 64], eye_sb[:])
            nc.scalar.copy(xsT[:, half * 512:(half + 1) * 512], ps_x[:])

        # mm1 + exp per (t-pair, half): logits[(q,m), n] in PSUM -> bf16 exp
        exps = {}
        for tp in range(6):
            for half in range(2):
                ps_log = psum.tile([128, 512], F32, tag="logits", bufs=2)
                nc.tensor.matmul(ps_log[:], mt2h_sb[:, tp * 128:(tp + 1) * 128],
                                 xsT[:, half * 512:(half + 1) * 512],
                                 start=True, stop=True)
                ex = work.tile([128, 512], BF16, tag="exp", bufs=24)
                nc.scalar.activation(ex[:], ps_log[:],
                                     mybir.ActivationFunctionType.Exp,
                                     bias=zbias[:])
                exps[(tp, half)] = ex

        # mm2 + normalize per 128-row chunk
        vn = work.tile([128, 8 * T * D], F16, tag="vn", bufs=2)
        for c in range(8):
            half, cc = divmod(c, 4)
            ps_val = psum.tile([128, 1024], F32, tag="val", bufs=2)
            for tp in range(6):
                off = 512 * (tp // 3) + 130 * (tp % 3)
                nc.tensor.matmul(ps_val[:, off:off + 130],
                                 exps[(tp, half)][:, cc * 128:(cc + 1) * 128],
                                 mbd_sb[:, tp * 130:(tp + 1) * 130],
                                 start=True, stop=True)
            # sums sit at free offsets {512h + 130a + 128 + q}; one strided
            # reciprocal covers all 12.
            sums_ap = (ps_val[:].rearrange("p (h r) -> p h r", h=2)
                       [:, :, 0:390]
                       .rearrange("p h (a r) -> p h a r", a=3)
                       [:, :, :, 128:130])
            rec = work.tile([128, 12], F32, tag="rec", bufs=4)
            nc.vector.reciprocal(
                rec[:].rearrange("p (h a t) -> p h a t", h=2, a=3), sums_ap)
            for h in range(2):
                in0 = (ps_val[:, 512 * h:512 * h + 390]
                       .rearrange("p (a r) -> p a r", a=3)
                       [:, :, 0:128]
                       .rearrange("p a (t d) -> p a t d", t=2))
                in1 = (rec[:, 6 * h:6 * h + 6]
                       .rearrange("p (a t) -> p a t", a=3)
                       .unsqueeze(3)
                       .broadcast_to([128, 3, 2, D]))
                outp = (vn[:, c * 768 + 384 * h:c * 768 + 384 * h + 384]
                        .rearrange("p (a t d) -> p a t d", a=3, t=2))
                nc.vector.tensor_mul(outp, in0, in1)
        # one 1.5 MB store on the ACT HWDGE ring (loads use the sync ring)
        nc.scalar.dma_start(
            out=out[r0:r0 + 1024, :].rearrange("(p c) f -> p c f", c=8),
            in_=vn[:].rearrange("p (c f) -> p c f", c=8),
        )


_NC_CACHE = {}


def build_nc():
    if "nc" in _NC_CACHE:
        return _NC_CACHE["nc"]
    nc = bacc.Bacc("TRN2", target_bir_lowering=False, debug=False,
                   num_devices=NCORES)
    # x is pre-transposed on the host to [BS, N, L, D], n-padded to 896 rows
    # per batch with zeros, flattened to [7168, 12, 64] and cast fp16. The
    # output is produced padded as [7168, (t d)] fp16; the host slices off
    # the 13 pad rows per batch and upcasts.
    x_ap = nc.dram_tensor("x_sh", [ROWS, L, D], F16, kind="ExternalInput").ap()
    mt2h_ap = nc.dram_tensor("mt2h", [64, 6 * 128], F16, kind="ExternalInput").ap()
    mbd_ap = nc.dram_tensor("mbd", [128, 6 * 130], BF16, kind="ExternalInput").ap()
    eye_ap = nc.dram_tensor("eye", [128, 128], F16, kind="ExternalInput").ap()
    out_ap = nc.dram_tensor("out", [ROWS, T * D], F16, kind="ExternalOutput").ap()
    with tile.TileContext(nc) as tc:
        kernel_body(tc, out_ap, x_ap, mt2h_ap, mbd_ap, eye_ap)
    nc.compile()
    _NC_CACHE["nc"] = nc
    return nc


def make_in_maps(x, M):
    import ml_dtypes
    x = np.asarray(x, dtype=np.float32)
    mt2h, mbd, eye = build_consts(M)
    mbd_bf = mbd.astype(ml_dtypes.bfloat16)
    maps = []
    for i in range(NCORES):
        xp = np.zeros((BS, NPAD, L, D), np.float16)
        xp[:, :N] = x[i * BS:(i + 1) * BS].transpose(0, 2, 1, 3).astype(np.float16)
        maps.append({"x_sh": xp.reshape(ROWS, L, D),
                     "mt2h": mt2h, "mbd": mbd_bf, "eye": eye})
    return maps


def gather_outputs(res):
    outs = []
    for i in range(NCORES):
        o = np.asarray(res[i]["out"], dtype=np.float32)
        o = o.reshape(BS, NPAD, T, D)[:, :N].transpose(0, 2, 1, 3)
        outs.append(o)
    return np.ascontiguousarray(np.concatenate(outs, axis=0))


def kernel(x, M):
    nc = build_nc()
    in_maps = make_in_maps(x, M)
    res = run_bass_kernel_spmd(nc, in_maps, list(range(NCORES))).results
    return gather_outputs(res)


if __name__ == "__main__":
    rng = np.random.default_rng(0)
    x = rng.standard_normal((B, L, N, D), dtype=np.float32)
    M = (rng.standard_normal((T, MNUM, D), dtype=np.float32) * 0.125).astype(np.float32)
    out = kernel(x, M)
    print("out", out.shape, out.dtype, float(np.abs(out).max()))


# revision 6
# speedup vs baseline: 1.1793x; 1.1272x over previous
"""Trainium2 Bass kernel for nn_MemoryAugmented (scatter_memory).

Computes, for full inputs x:[64,12,883,64], M:[12,64,64]:
    score = softmax(einsum('blnd,tmd->btnm', x, M), axis=-1)
    out   = einsum('btnm,tmd->btnd', score, M)

Distribution: data-parallel over batch across 8 NeuronCores (8 batches
per core); the memory bank M is replicated, shipped pre-transformed into
two constant matrices (paired-t M^T for mm1, block-diagonal M + ones
columns for mm2's fused row sums).

Precision: x and M travel as fp16 (matmuls run at 1 cycle/row vs 4 for
fp32, HBM traffic halves); exp values are bf16 (need fp32-like range);
PSUM accumulation is always fp32; output is stored fp16 and upcast on
the host. Measured end-to-end max rel err ~4e-3 vs the 2e-2 gate.

Per-core dataflow, 7 iterations of 1024 rows r = (b, n), software-
pipelined one deep so no engine waits on another's latest result:
  body(it): load(it+1); l-sum tree(it+1) [L1 on gpsimd, L2-4 on DVE];
  for each t-pair: mm1(it) x2 + exp(it) x2 interleaved with one value
  chunk of mm2(it-1) + reciprocal + normalize (DVE); remaining chunks;
  store(it-1); PE transposes + ACT copies build xsT(it+1) last.
"""
import sys

for _p in ("/opt/trn_rl_repo",):
    if _p not in sys.path:
        sys.path.insert(0, _p)

from contextlib import ExitStack

import numpy as np

import concourse.bass as bass
import concourse.bacc as bacc
import concourse.tile as tile
from concourse import mybir
from concourse._compat import with_exitstack
from concourse.bass_utils import run_bass_kernel_spmd

B, L, N, D = 64, 12, 883, 64
T, MNUM = 12, 64
NCORES = 8
BS = B // NCORES          # 8 batches per core
NPAD = 896                # per-batch row pad (7*128)
ROWS = BS * NPAD          # 7168 rows per core
NIT = 7                   # iterations of 1024 rows
F32 = mybir.dt.float32
F16 = mybir.dt.float16
BF16 = mybir.dt.bfloat16


def build_consts(M):
    """Host-side layout prep (pure data movement) of the memory bank."""
    M = np.asarray(M, dtype=np.float32)
    mt2h = np.zeros((64, 6 * 128), np.float16)   # [d, (tp, q, m)] = M[2tp+q].T
    mbd = np.zeros((128, 6 * 130), np.float32)   # [(q, m), (tp, q, d | sums)]
    for tp in range(6):
        t0, t1 = 2 * tp, 2 * tp + 1
        mt2h[:, tp * 128 + 0:tp * 128 + 64] = M[t0].T.astype(np.float16)
        mt2h[:, tp * 128 + 64:tp * 128 + 128] = M[t1].T.astype(np.float16)
        mbd[0:64, tp * 130 + 0:tp * 130 + 64] = M[t0]
        mbd[64:128, tp * 130 + 64:tp * 130 + 128] = M[t1]
        mbd[0:64, tp * 130 + 128] = 1.0
        mbd[64:128, tp * 130 + 129] = 1.0
    eye = np.eye(128, dtype=np.float16)
    return mt2h, mbd, eye


@with_exitstack
def kernel_body(ctx: ExitStack, tc: "tile.TileContext", out: bass.AP,
                x: bass.AP, mt2h: bass.AP, mbd: bass.AP, eye: bass.AP):
    nc = tc.nc
    consts = ctx.enter_context(tc.tile_pool(name="consts", bufs=1))
    work = ctx.enter_context(tc.tile_pool(name="work", bufs=2))
    psum = ctx.enter_context(tc.tile_pool(name="psum", bufs=1, space="PSUM"))

    # const loads ride the scalar HWDGE ring (idle at kernel start) so the
    # first x-load isn't queued behind them on the sync ring's FIFO.
    mt2h_sb = consts.tile([64, 6 * 128], F16)
    nc.scalar.dma_start(out=mt2h_sb[:], in_=mt2h[:])
    mbd_sb = consts.tile([128, 6 * 130], BF16)
    nc.scalar.dma_start(out=mbd_sb[:], in_=mbd[:])
    eye_sb = consts.tile([128, 128], F16)
    nc.scalar.dma_start(out=eye_sb[:], in_=eye[:])
    zbias = consts.tile([128, 1], F32)
    nc.vector.memset(zbias[:], 0.0)

    def load(it):
        # one 1.5 MB load; partition p <- rows 8p..8p+7 (12 KB contiguous)
        xt = work.tile([128, 8 * L * D], F16, tag="xt", bufs=2)
        nc.sync.dma_start(
            out=xt[:].rearrange("p (c f) -> p c f", c=8),
            in_=x[1024 * it:1024 * it + 1024, :, :]
                .rearrange("(p c) l d -> p c (l d)", c=8),
        )
        return xt

    def tree_l1(xt):
        # 12 -> 6 on gpsimd (otherwise idle); the rest on DVE in fp16 2x
        t384 = work.tile([128, 8 * 384], F16, tag="t384", bufs=2)
        xtv = xt[:].rearrange("p (c h f) -> p c h f", c=8, h=2)
        nc.gpsimd.tensor_add(t384[:].rearrange("p (c f) -> p c f", c=8),
                             xtv[:, :, 0], xtv[:, :, 1])
        return t384

    def tree_rest(t384):
        t192 = work.tile([128, 8 * 192], F16, tag="t192", bufs=2)
        t384v = t384[:].rearrange("p (c h f) -> p c h f", c=8, h=2)
        nc.vector.tensor_add(t192[:].rearrange("p (c f) -> p c f", c=8),
                             t384v[:, :, 0], t384v[:, :, 1])
        t192v = t192[:].rearrange("p (c g f) -> p c g f", c=8, g=3)
        xs2 = work.tile([128, 8 * 64], F16, tag="xs2", bufs=2)
        xs2v = xs2[:].rearrange("p (c f) -> p c f", c=8)
        nc.vector.tensor_add(xs2v, t192v[:, :, 0], t192v[:, :, 1])
        xs4 = work.tile([128, 8 * 64], F16, tag="xs4", bufs=2)
        nc.vector.tensor_add(xs4[:].rearrange("p (c f) -> p c f", c=8),
                             xs2v, t192v[:, :, 2])
        return xs4

    def build_xsT(xs4):
        # transpose the 8 chunks to xsT [64 d, 1024 n] via PE + ACT copy
        xsT = work.tile([64, 1024], F16, tag="xsT", bufs=2)
        for half in range(2):
            ps_x = psum.tile([64, 512], F16, tag="ps_x", bufs=2)
            for cc in range(4):
                c = 4 * half + cc
                nc.tensor.transpose(ps_x[:, cc * 128:(cc + 1) * 128],
                                    xs4[:, c * 64:(c + 1) * 64], eye_sb[:])
            nc.scalar.copy(xsT[:, half * 512:(half + 1) * 512], ps_x[:])
        return xsT

    def mm1_exp(xsT, tp):
        pair = []
        for half in range(2):
            ps_log = psum.tile([128, 512], F32, tag="logits", bufs=2)
            nc.tensor.matmul(ps_log[:], mt2h_sb[:, tp * 128:(tp + 1) * 128],
                             xsT[:, half * 512:(half + 1) * 512],
                             start=True, stop=True)
            ex = work.tile([128, 512], BF16, tag="exp", bufs=24)
            nc.scalar.activation(ex[:], ps_log[:],
                                 mybir.ActivationFunctionType.Exp,
                                 bias=zbias[:])
            pair.append(ex)
        return pair

    def chunk_val(exps, vn, c):
        half, cc = divmod(c, 4)
        ps_val = psum.tile([128, 1024], F32, tag="val", bufs=2)
        for tp in range(6):
            off = 512 * (tp // 3) + 130 * (tp % 3)
            nc.tensor.matmul(ps_val[:, off:off + 130],
                             exps[(tp, half)][:, cc * 128:(cc + 1) * 128],
                             mbd_sb[:, tp * 130:(tp + 1) * 130],
                             start=True, stop=True)
        sums_ap = (ps_val[:].rearrange("p (h r) -> p h r", h=2)
                   [:, :, 0:390]
                   .rearrange("p h (a r) -> p h a r", a=3)
                   [:, :, :, 128:130])
        rec = work.tile([128, 12], F32, tag="rec", bufs=4)
        nc.vector.reciprocal(
            rec[:].rearrange("p (h a t) -> p h a t", h=2, a=3), sums_ap)
        for h in range(2):
            in0 = (ps_val[:, 512 * h:512 * h + 390]
                   .rearrange("p (a r) -> p a r", a=3)
                   [:, :, 0:128]
                   .rearrange("p a (t d) -> p a t d", t=2))
            in1 = (rec[:, 6 * h:6 * h + 6]
                   .rearrange("p (a t) -> p a t", a=3)
                   .unsqueeze(3)
                   .broadcast_to([128, 3, 2, D]))
            outp = (vn[:, c * 768 + 384 * h:c * 768 + 384 * h + 384]
                    .rearrange("p (a t d) -> p a t d", a=3, t=2))
            nc.vector.tensor_mul(outp, in0, in1)

    def store(it, vn):
        # 1.5 MB store on the ACT HWDGE ring (loads use the sync ring)
        nc.scalar.dma_start(
            out=out[1024 * it:1024 * it + 1024, :]
                .rearrange("(p c) f -> p c f", c=8),
            in_=vn[:].rearrange("p (c f) -> p c f", c=8),
        )

    # -------- prologue: get iteration 0's xsT ready --------
    xt0 = load(0)
    xsT = build_xsT(tree_rest(tree_l1(xt0)))

    exps_prev = None
    vn_prev = None
    for it in range(NIT + 1):
        t384_next = None
        if it < NIT - 1:
            xt = load(it + 1)
            t384_next = tree_l1(xt)          # gpsimd starts early
        exps = {}
        vn = None
        if it < NIT:
            vn = work.tile([128, 8 * T * D], F16, tag="vn", bufs=2)
        # interleave this iteration's mm1/exp pairs with the previous
        # iteration's mm2 chunks so PE never idles on ACT's exp pace.
        for tp in range(6):
            if it < NIT:
                exps[(tp, 0)], exps[(tp, 1)] = mm1_exp(xsT, tp)
            if it > 0:
                chunk_val(exps_prev, vn_prev, tp)
        if it > 0:
            # finish DVE's tree for it+1 before chunks 6/7 so L4 is ready
            # when PE reaches the transposes below.
            if t384_next is not None:
                xs4_next = tree_rest(t384_next)
            chunk_val(exps_prev, vn_prev, 6)
            chunk_val(exps_prev, vn_prev, 7)
            store(it - 1, vn_prev)
        else:
            if t384_next is not None:
                xs4_next = tree_rest(t384_next)
        if it < NIT - 1:
            xsT = build_xsT(xs4_next)
        exps_prev, vn_prev = exps, vn


_NC_CACHE = {}


def build_nc():
    if "nc" in _NC_CACHE:
        return _NC_CACHE["nc"]
    nc = bacc.Bacc("TRN2", target_bir_lowering=False, debug=False,
                   num_devices=NCORES)
    # x is pre-transposed on the host to [BS, N, L, D], n-padded to 896 rows
    # per batch with zeros, flattened to [7168, 12, 64] and cast fp16. The
    # output is produced padded as [7168, (t d)] fp16; the host slices off
    # the 13 pad rows per batch and upcasts.
    x_ap = nc.dram_tensor("x_sh", [ROWS, L, D], F16, kind="ExternalInput").ap()
    mt2h_ap = nc.dram_tensor("mt2h", [64, 6 * 128], F16, kind="ExternalInput").ap()
    mbd_ap = nc.dram_tensor("mbd", [128, 6 * 130], BF16, kind="ExternalInput").ap()
    eye_ap = nc.dram_tensor("eye", [128, 128], F16, kind="ExternalInput").ap()
    out_ap = nc.dram_tensor("out", [ROWS, T * D], F16, kind="ExternalOutput").ap()
    with tile.TileContext(nc) as tc:
        kernel_body(tc, out_ap, x_ap, mt2h_ap, mbd_ap, eye_ap)
    nc.compile()
    _NC_CACHE["nc"] = nc
    return nc


def make_in_maps(x, M):
    import ml_dtypes
    x = np.asarray(x, dtype=np.float32)
    mt2h, mbd, eye = build_consts(M)
    mbd_bf = mbd.astype(ml_dtypes.bfloat16)
    maps = []
    for i in range(NCORES):
        xp = np.zeros((BS, NPAD, L, D), np.float16)
        xp[:, :N] = x[i * BS:(i + 1) * BS].transpose(0, 2, 1, 3).astype(np.float16)
        maps.append({"x_sh": xp.reshape(ROWS, L, D),
                     "mt2h": mt2h, "mbd": mbd_bf, "eye": eye})
    return maps


def gather_outputs(res):
    outs = []
    for i in range(NCORES):
        o = np.asarray(res[i]["out"], dtype=np.float32)
        o = o.reshape(BS, NPAD, T, D)[:, :N].transpose(0, 2, 1, 3)
        outs.append(o)
    return np.ascontiguousarray(np.concatenate(outs, axis=0))


def kernel(x, M):
    nc = build_nc()
    in_maps = make_in_maps(x, M)
    res = run_bass_kernel_spmd(nc, in_maps, list(range(NCORES))).results
    return gather_outputs(res)


if __name__ == "__main__":
    rng = np.random.default_rng(0)
    x = rng.standard_normal((B, L, N, D), dtype=np.float32)
    M = (rng.standard_normal((T, MNUM, D), dtype=np.float32) * 0.125).astype(np.float32)
    out = kernel(x, M)
    print("out", out.shape, out.dtype, float(np.abs(out).max()))
